# revision 3
# baseline (speedup 1.0000x reference)
"""Trainium kernel for nn_AttentionLayerO2TwoUpdateNodeGeneral_cross.

Strategy (sharding hint: shard along the graph/batch dimension):
  - B=32 complexes, each a dense bipartite graph of 350 protein x 30 ligand
    atoms. Edges per graph are the full 350x30 grid in p-major order
    (dst = protein repeated NL times, src = ligand tiled), so every
    gather/scatter in the reference collapses to dense reshapes/reductions
    over a [350, 30] grid that lives entirely on one core.
  - 8 NeuronCores, 4 complexes per core; MLP weights replicated.
  - The scatter-softmax groups by src (ligand nodes): a softmax over the
    350 protein neighbors for each (ligand, head). Aggregated messages only
    touch ligand rows; protein rows receive a single shared constant vector
    c = MLP_no([0, h[0]]) because segment_sum is zero there and
    h[mask_ligand] is row 0 of h for every protein atom (int mask quirk).
  - q = MLP_hq(h) is only consumed at q[src], i.e. ligand rows: computed for
    the 30 ligand atoms per graph instead of all 12160 nodes.

Everything runs in fp32 on the 8 neuron cores via a single pmap'd program.
"""

import numpy as np

# ---- static dims (hardcoded per problem spec) ----
B, NP_, NL = 32, 350, 30
NG = NP_ + NL            # 380
N = B * NG               # 12160
HID, HEADS, G = 128, 16, 20
DH = HID // HEADS        # 8
KV = 2 * HID + G         # 276
R_MIN, R_MAX = 0.0, 10.0
NCORES = 8
GPC = B // NCORES        # 4 graphs per core

_COMPILED = {}


def _build():
    import jax
    import jax.numpy as jnp

    offset = np.linspace(R_MIN, R_MAX, G).astype(np.float32)
    coeff = np.float32(-0.5 / (offset[1] - offset[0]) ** 2)
    inv_sqrt_dh = np.float32(1.0 / np.sqrt(DH))

    def mlp(t, p):
        w1, b1, g, be, w2, b2 = p
        y = t @ w1 + b1
        mu = y.mean(-1, keepdims=True)
        var = ((y - mu) ** 2).mean(-1, keepdims=True)
        y = jax.nn.relu((y - mu) * jax.lax.rsqrt(var + 1e-5) * g + be)
        return y @ w2 + b2

    def edge_mlp(r_feat, hp, hl, p):
        # kv = [r_feat(20), h_prot(128), h_lig(128)] without materializing it:
        # kv @ w1 = r_feat @ w1[:20] + hp @ w1[20:148] + hl @ w1[148:276]
        w1, b1, g, be, w2, b2 = p
        A = hp @ w1[G:G + HID]                       # [g,350,Hm]
        Bl = hl @ w1[G + HID:]                       # [g,30,Hm]
        yr = (r_feat.reshape(-1, G) @ w1[:G]).reshape(GPC, NP_, NL, HID)
        y = yr + A[:, :, None, :] + Bl[:, None, :, :] + b1
        mu = y.mean(-1, keepdims=True)
        var = ((y - mu) ** 2).mean(-1, keepdims=True)
        z = jax.nn.relu((y - mu) * jax.lax.rsqrt(var + 1e-5) * g + be)
        out = z.reshape(-1, HID) @ w2 + b2
        return out.reshape(GPC, NP_, NL, -1)

    def per_core(h, x, h01, *ws):
        # h: [GPC, NG, HID], x: [GPC, NG, 3], h01: [2, HID] (global rows 0/1)
        pk, pv, pxv, pq, pno = (ws[i * 6:(i + 1) * 6] for i in range(5))
        hp, hl = h[:, :NP_], h[:, NP_:]          # [g,350,128], [g,30,128]
        xp, xl = x[:, :NP_], x[:, NP_:]
        # rel_x = x[src] - x[dst] = x_lig - x_prot, on the [p, l] grid
        rel = xl[:, None, :, :] - xp[:, :, None, :]          # [g,350,30,3]
        dist = jnp.sqrt(jnp.sum(rel * rel, -1, keepdims=True))  # [g,350,30,1]
        r_feat = jnp.exp(coeff * (dist - offset) ** 2)          # [g,350,30,20]
        k = edge_mlp(r_feat, hp, hl, pk).reshape(GPC, NP_, NL, HEADS, DH)
        v = edge_mlp(r_feat, hp, hl, pv).reshape(GPC, NP_, NL, HEADS, DH)
        xvv = edge_mlp(r_feat, hp, hl, pxv)[..., 0:HEADS]      # [g,350,30,16]
        q = mlp(hl, pq).reshape(GPC, NL, HEADS, DH)            # ligand only
        logits = (q[:, None, :, :, :] * k).sum(-1) * inv_sqrt_dh
        # softmax over protein axis p (the per-src segments)
        m = logits.max(1, keepdims=True)
        ex = jnp.exp(logits - m)
        alpha = ex / ex.sum(1, keepdims=True)                  # [g,350,30,16]
        out_l = (alpha[..., None] * v).sum(1).reshape(GPC, NL, HID)
        axv = alpha * xvv
        out_x_l = (axv[..., None] * rel[:, :, :, None, :]).sum(1)  # [g,30,16,3]
        # out_fc on ligand rows: h[mask_ligand]=h[1] (global row 1)
        h1 = jnp.broadcast_to(h01[1], (GPC, NL, HID))
        h_out_l = mlp(jnp.concatenate([out_l, h1], -1), pno) + hl
        x_out_l = xl + out_x_l.mean(2)
        # protein rows: shared constant c = MLP_no([0, h[0]])
        cvec = mlp(jnp.concatenate([jnp.zeros((HID,), jnp.float32), h01[0]]), pno)
        h_out_p = hp + cvec
        h_out = jnp.concatenate([h_out_p, h_out_l], 1)         # [g,380,128]
        x_out = jnp.concatenate([xp, x_out_l], 1)              # [g,380,3]
        return h_out, x_out

    fn = jax.pmap(
        per_core,
        axis_name="c",
        in_axes=(0, 0) + (None,) * (2 + 30 - 1),
        devices=jax.devices()[:NCORES],
    )
    return fn


def kernel(h, x, src, dst, mask_ligand,
           hk_w1, hk_b1, hk_g, hk_be, hk_w2, hk_b2,
           hv_w1, hv_b1, hv_g, hv_be, hv_w2, hv_b2,
           xv_w1, xv_b1, xv_g, xv_be, xv_w2, xv_b2,
           hq_w1, hq_b1, hq_g, hq_be, hq_w2, hq_b2,
           no_w1, no_b1, no_g, no_be, no_w2, no_b2):
    if "fn" not in _COMPILED:
        _COMPILED["fn"] = _build()
    fn = _COMPILED["fn"]

    h = np.asarray(h, np.float32)
    x = np.asarray(x, np.float32)
    hs = h.reshape(NCORES, GPC, NG, HID)
    xs = x.reshape(NCORES, GPC, NG, 3)
    h01 = h[:2]  # global rows 0 and 1 (int-mask gather quirk)
    ws = [np.asarray(w, np.float32) for w in (
        hk_w1, hk_b1, hk_g, hk_be, hk_w2, hk_b2,
        hv_w1, hv_b1, hv_g, hv_be, hv_w2, hv_b2,
        xv_w1, xv_b1, xv_g, xv_be, xv_w2, xv_b2,
        hq_w1, hq_b1, hq_g, hq_be, hq_w2, hq_b2,
        no_w1, no_b1, no_g, no_be, no_w2, no_b2)]
    h_out, x_out = fn(hs, xs, h01, *ws)
    h_out = np.asarray(h_out).reshape(N, HID).astype(np.float32)
    x_out = np.asarray(x_out).reshape(N, 3).astype(np.float32)
    return h_out, x_out


# revision 5
# speedup vs baseline: 1.1961x; 1.1961x over previous
"""Trainium kernel for nn_AttentionLayerO2TwoUpdateNodeGeneral_cross.

Strategy (sharding hint: shard along the graph/batch dimension):
  - B=32 complexes, each a dense bipartite graph of 350 protein x 30 ligand
    atoms. Edges per graph are the full 350x30 grid in p-major order
    (dst = protein repeated NL times, src = ligand tiled), so every
    gather/scatter in the reference collapses to dense reshapes/reductions
    over a [350, 30] grid that lives entirely on one core.
  - 8 NeuronCores, 4 complexes per core; MLP weights replicated.
  - The scatter-softmax groups by src (ligand nodes): a softmax over the
    350 protein neighbors for each (ligand, head). Aggregated messages only
    touch ligand rows; protein rows receive a single shared constant vector
    c = MLP_no([0, h[0]]) because segment_sum is zero there and
    h[mask_ligand] is row 0 of h for every protein atom (int mask quirk).
  - q = MLP_hq(h) is only consumed at q[src], i.e. ligand rows: computed for
    the 30 ligand atoms per graph instead of all 12160 nodes.

Everything runs in fp32 on the 8 neuron cores via a single pmap'd program.
"""

import numpy as np

# ---- static dims (hardcoded per problem spec) ----
B, NP_, NL = 32, 350, 30
NG = NP_ + NL            # 380
N = B * NG               # 12160
HID, HEADS, G = 128, 16, 20
DH = HID // HEADS        # 8
KV = 2 * HID + G         # 276
R_MIN, R_MAX = 0.0, 10.0
NCORES = 8
GPC = B // NCORES        # 4 graphs per core

_COMPILED = {}


def _build():
    import jax
    import jax.numpy as jnp

    offset = np.linspace(R_MIN, R_MAX, G).astype(np.float32)
    coeff = np.float32(-0.5 / (offset[1] - offset[0]) ** 2)
    inv_sqrt_dh = np.float32(1.0 / np.sqrt(DH))

    def mlp(t, p):
        w1, b1, g, be, w2, b2 = p
        y = t @ w1 + b1
        mu = y.mean(-1, keepdims=True)
        var = ((y - mu) ** 2).mean(-1, keepdims=True)
        y = jax.nn.relu((y - mu) * jax.lax.rsqrt(var + 1e-5) * g + be)
        return y @ w2 + b2

    def edge_mlp(r_feat, hp, hl, p):
        # kv = [r_feat(20), h_prot(128), h_lig(128)] without materializing it:
        # kv @ w1 = r_feat @ w1[:20] + hp @ w1[20:148] + hl @ w1[148:276]
        w1, b1, g, be, w2, b2 = p
        A = hp @ w1[G:G + HID]                       # [g,350,Hm]
        Bl = hl @ w1[G + HID:]                       # [g,30,Hm]
        yr = (r_feat.reshape(-1, G) @ w1[:G]).reshape(GPC, NP_, NL, HID)
        y = yr + A[:, :, None, :] + Bl[:, None, :, :] + b1
        mu = y.mean(-1, keepdims=True)
        var = ((y - mu) ** 2).mean(-1, keepdims=True)
        z = jax.nn.relu((y - mu) * jax.lax.rsqrt(var + 1e-5) * g + be)
        out = z.reshape(-1, HID) @ w2 + b2
        return out.reshape(GPC, NP_, NL, -1)

    def per_core(h, x, h01, *ws):
        # h: [GPC, NG, HID], x: [GPC, NG, 3], h01: [2, HID] (global rows 0/1)
        pk, pv, pxv, pq, pno = (ws[i * 6:(i + 1) * 6] for i in range(5))
        hp, hl = h[:, :NP_], h[:, NP_:]          # [g,350,128], [g,30,128]
        xp, xl = x[:, :NP_], x[:, NP_:]
        # rel_x = x[src] - x[dst] = x_lig - x_prot, on the [p, l] grid
        rel = xl[:, None, :, :] - xp[:, :, None, :]          # [g,350,30,3]
        dist = jnp.sqrt(jnp.sum(rel * rel, -1, keepdims=True))  # [g,350,30,1]
        r_feat = jnp.exp(coeff * (dist - offset) ** 2)          # [g,350,30,20]
        k = edge_mlp(r_feat, hp, hl, pk)                       # [g,350,30,128]
        v = edge_mlp(r_feat, hp, hl, pv)                       # [g,350,30,128]
        xvv = edge_mlp(r_feat, hp, hl, pxv)                    # [g,350,30,16]
        q = mlp(hl, pq)                                        # [g,30,128]
        # logits[g,p,l,h] = sum_d q[g,l,h*8+d] * k[g,p,l,h*8+d] / sqrt(8)
        qk = k * q[:, None, :, :]                              # [g,350,30,128]
        logits = qk.reshape(GPC, NP_, NL, HEADS, DH).sum(-1) * inv_sqrt_dh
        # softmax over protein axis p (the per-src segments)
        m = logits.max(1, keepdims=True)
        ex = jnp.exp(logits - m)
        alpha = ex / ex.sum(1, keepdims=True)                  # [g,350,30,16]
        # broadcast alpha over the 8 dims of each head without a transpose
        alpha128 = jnp.broadcast_to(
            alpha[..., None], (GPC, NP_, NL, HEADS, DH)
        ).reshape(GPC, NP_, NL, HID)
        out_l = (alpha128 * v).sum(1).reshape(GPC, NL, HID)
        axv = alpha * xvv                                      # [g,350,30,16]
        out_x_l = (axv[..., None] * rel[:, :, :, None, :]).sum(1)  # [g,30,16,3]
        # out_fc on ligand rows: h[mask_ligand]=h[1] (global row 1)
        h1 = jnp.broadcast_to(h01[1], (GPC, NL, HID))
        h_out_l = mlp(jnp.concatenate([out_l, h1], -1), pno) + hl
        x_out_l = xl + out_x_l.mean(2)
        # protein rows: shared constant c = MLP_no([0, h[0]])
        cvec = mlp(jnp.concatenate([jnp.zeros((HID,), jnp.float32), h01[0]]), pno)
        h_out_p = hp + cvec
        h_out = jnp.concatenate([h_out_p, h_out_l], 1)         # [g,380,128]
        x_out = jnp.concatenate([xp, x_out_l], 1)              # [g,380,3]
        return h_out, x_out

    fn = jax.pmap(
        per_core,
        axis_name="c",
        in_axes=(0, 0) + (None,) * (2 + 30 - 1),
        devices=jax.devices()[:NCORES],
    )
    return fn


def kernel(h, x, src, dst, mask_ligand,
           hk_w1, hk_b1, hk_g, hk_be, hk_w2, hk_b2,
           hv_w1, hv_b1, hv_g, hv_be, hv_w2, hv_b2,
           xv_w1, xv_b1, xv_g, xv_be, xv_w2, xv_b2,
           hq_w1, hq_b1, hq_g, hq_be, hq_w2, hq_b2,
           no_w1, no_b1, no_g, no_be, no_w2, no_b2):
    import jax

    if "fn" not in _COMPILED:
        _COMPILED["fn"] = _build()
    fn = _COMPILED["fn"]

    h = np.asarray(h, np.float32)
    x = np.asarray(x, np.float32)
    hs = h.reshape(NCORES, GPC, NG, HID)
    xs = x.reshape(NCORES, GPC, NG, 3)
    h01 = h[:2]  # global rows 0 and 1 (int-mask gather quirk)
    ws = [np.asarray(w, np.float32) for w in (
        hk_w1, hk_b1, hk_g, hk_be, hk_w2, hk_b2,
        hv_w1, hv_b1, hv_g, hv_be, hv_w2, hv_b2,
        xv_w1, xv_b1, xv_g, xv_be, xv_w2, xv_b2,
        hq_w1, hq_b1, hq_g, hq_be, hq_w2, hq_b2,
        no_w1, no_b1, no_g, no_be, no_w2, no_b2)]
    devs = jax.devices()[:NCORES]
    hs_d = jax.device_put_sharded([hs[i] for i in range(NCORES)], devs)
    xs_d = jax.device_put_sharded([xs[i] for i in range(NCORES)], devs)
    h_out, x_out = fn(hs_d, xs_d, h01, *ws)
    h_out = np.asarray(h_out).reshape(N, HID).astype(np.float32)
    x_out = np.asarray(x_out).reshape(N, 3).astype(np.float32)
    return h_out, x_out


# revision 7
# speedup vs baseline: 1.8715x; 1.5646x over previous
"""Trainium kernel for nn_AttentionLayerO2TwoUpdateNodeGeneral_cross.

Strategy (sharding hint: shard along the graph/batch dimension):
  - B=32 complexes, each a dense bipartite graph of 350 protein x 30 ligand
    atoms. Edges per graph are the full 350x30 grid in p-major order
    (dst = protein repeated NL times, src = ligand tiled), so every
    gather/scatter in the reference collapses to dense reshapes/reductions
    over a [350, 30] grid that lives entirely on one core.
  - 8 NeuronCores, 4 complexes per core; MLP weights replicated.
  - The scatter-softmax groups by src (ligand nodes): a softmax over the
    350 protein neighbors for each (ligand, head). Aggregated messages only
    touch ligand rows; protein rows receive a single shared constant vector
    c = MLP_no([0, h[0]]) because segment_sum is zero there and
    h[mask_ligand] is row 0 of h for every protein atom (int mask quirk).
  - q = MLP_hq(h) is only consumed at q[src], i.e. ligand rows: computed for
    the 30 ligand atoms per graph instead of all 12160 nodes.

Everything runs in fp32 on the 8 neuron cores via a single pmap'd program.
"""

import numpy as np

# ---- static dims (hardcoded per problem spec) ----
B, NP_, NL = 32, 350, 30
NG = NP_ + NL            # 380
N = B * NG               # 12160
HID, HEADS, G = 128, 16, 20
DH = HID // HEADS        # 8
KV = 2 * HID + G         # 276
R_MIN, R_MAX = 0.0, 10.0
NCORES = 8
GPC = B // NCORES        # 4 graphs per core

_COMPILED = {}


def _build():
    import jax
    import jax.numpy as jnp

    offset = np.linspace(R_MIN, R_MAX, G).astype(np.float32)
    coeff = np.float32(-0.5 / (offset[1] - offset[0]) ** 2)
    inv_sqrt_dh = np.float32(1.0 / np.sqrt(DH))

    def mlp(t, p):
        w1, b1, g, be, w2, b2 = p
        y = t @ w1 + b1
        mu = y.mean(-1, keepdims=True)
        var = ((y - mu) ** 2).mean(-1, keepdims=True)
        y = jax.nn.relu((y - mu) * jax.lax.rsqrt(var + 1e-5) * g + be)
        return y @ w2 + b2

    def edge_mlp(r_feat, hp, hl, p):
        # kv = [r_feat(20), h_prot(128), h_lig(128)] without materializing it:
        # kv @ w1 = r_feat @ w1[:20] + hp @ w1[20:148] + hl @ w1[148:276]
        w1, b1, g, be, w2, b2 = p
        A = hp @ w1[G:G + HID]                       # [g,350,Hm]
        Bl = hl @ w1[G + HID:]                       # [g,30,Hm]
        yr = (r_feat.reshape(-1, G) @ w1[:G]).reshape(GPC, NP_, NL, HID)
        y = yr + A[:, :, None, :] + Bl[:, None, :, :] + b1
        mu = y.mean(-1, keepdims=True)
        var = ((y - mu) ** 2).mean(-1, keepdims=True)
        z = jax.nn.relu((y - mu) * jax.lax.rsqrt(var + 1e-5) * g + be)
        out = z.reshape(-1, HID) @ w2 + b2
        return out.reshape(GPC, NP_, NL, -1)

    def per_core(h, x, h01, *ws):
        # h: [GPC, NG, HID], x: [GPC, NG, 3], h01: [2, HID] (global rows 0/1)
        pk, pv, pxv, pq, pno = (ws[i * 6:(i + 1) * 6] for i in range(5))
        hp, hl = h[:, :NP_], h[:, NP_:]          # [g,350,128], [g,30,128]
        xp, xl = x[:, :NP_], x[:, NP_:]
        # rel_x = x[src] - x[dst] = x_lig - x_prot, on the [p, l] grid
        rel = xl[:, None, :, :] - xp[:, :, None, :]          # [g,350,30,3]
        dist = jnp.sqrt(jnp.sum(rel * rel, -1, keepdims=True))  # [g,350,30,1]
        r_feat = jnp.exp(coeff * (dist - offset) ** 2)          # [g,350,30,20]
        k = edge_mlp(r_feat, hp, hl, pk)                       # [g,350,30,128]
        v = edge_mlp(r_feat, hp, hl, pv)                       # [g,350,30,128]
        xvv = edge_mlp(r_feat, hp, hl, pxv)                    # [g,350,30,16]
        q = mlp(hl, pq)                                        # [g,30,128]
        # logits[g,p,l,h] = sum_d q[g,l,h*8+d] * k[g,p,l,h*8+d] / sqrt(8)
        qk = k * q[:, None, :, :]                              # [g,350,30,128]
        logits = qk.reshape(GPC, NP_, NL, HEADS, DH).sum(-1) * inv_sqrt_dh
        # softmax over protein axis p (the per-src segments)
        m = logits.max(1, keepdims=True)
        ex = jnp.exp(logits - m)
        alpha = ex / ex.sum(1, keepdims=True)                  # [g,350,30,16]
        # broadcast alpha over the 8 dims of each head without a transpose
        alpha128 = jnp.broadcast_to(
            alpha[..., None], (GPC, NP_, NL, HEADS, DH)
        ).reshape(GPC, NP_, NL, HID)
        out_l = (alpha128 * v).sum(1).reshape(GPC, NL, HID)
        axv = alpha * xvv                                      # [g,350,30,16]
        out_x_l = (axv[..., None] * rel[:, :, :, None, :]).sum(1)  # [g,30,16,3]
        # out_fc on ligand rows: h[mask_ligand]=h[1] (global row 1)
        h1 = jnp.broadcast_to(h01[1], (GPC, NL, HID))
        h_out_l = mlp(jnp.concatenate([out_l, h1], -1), pno) + hl
        x_out_l = xl + out_x_l.mean(2)
        # protein rows: shared constant c = MLP_no([0, h[0]])
        cvec = mlp(jnp.concatenate([jnp.zeros((HID,), jnp.float32), h01[0]]), pno)
        h_out_p = hp + cvec
        h_out = jnp.concatenate([h_out_p, h_out_l], 1)         # [g,380,128]
        x_out = jnp.concatenate([xp, x_out_l], 1)              # [g,380,3]
        return h_out, x_out

    fn = jax.pmap(
        per_core,
        axis_name="c",
        in_axes=(0,) * 33,
        devices=jax.devices()[:NCORES],
    )
    return fn


def kernel(h, x, src, dst, mask_ligand,
           hk_w1, hk_b1, hk_g, hk_be, hk_w2, hk_b2,
           hv_w1, hv_b1, hv_g, hv_be, hv_w2, hv_b2,
           xv_w1, xv_b1, xv_g, xv_be, xv_w2, xv_b2,
           hq_w1, hq_b1, hq_g, hq_be, hq_w2, hq_b2,
           no_w1, no_b1, no_g, no_be, no_w2, no_b2):
    import jax

    if "fn" not in _COMPILED:
        _COMPILED["fn"] = _build()
    fn = _COMPILED["fn"]

    h = np.asarray(h, np.float32)
    x = np.asarray(x, np.float32)
    hs = h.reshape(NCORES, GPC, NG, HID)
    xs = x.reshape(NCORES, GPC, NG, 3)
    h01 = h[:2]  # global rows 0 and 1 (int-mask gather quirk)
    ws = [np.asarray(w, np.float32) for w in (
        hk_w1, hk_b1, hk_g, hk_be, hk_w2, hk_b2,
        hv_w1, hv_b1, hv_g, hv_be, hv_w2, hv_b2,
        xv_w1, xv_b1, xv_g, xv_be, xv_w2, xv_b2,
        hq_w1, hq_b1, hq_g, hq_be, hq_w2, hq_b2,
        no_w1, no_b1, no_g, no_be, no_w2, no_b2)]
    devs = jax.devices()[:NCORES]
    hs_d = jax.device_put_sharded([hs[i] for i in range(NCORES)], devs)
    xs_d = jax.device_put_sharded([xs[i] for i in range(NCORES)], devs)
    h01_d = jax.device_put_replicated(h01, devs)
    # replicate weights once; reuse across calls when contents unchanged
    import hashlib
    wkey = hashlib.md5(b"".join(w.tobytes() for w in ws)).hexdigest()
    if _COMPILED.get("wkey") != wkey:
        _COMPILED["ws_d"] = [jax.device_put_replicated(w, devs) for w in ws]
        _COMPILED["wkey"] = wkey
    h_out, x_out = fn(hs_d, xs_d, h01_d, *_COMPILED["ws_d"])
    h_out = np.asarray(h_out).reshape(N, HID).astype(np.float32)
    x_out = np.asarray(x_out).reshape(N, 3).astype(np.float32)
    return h_out, x_out
